# revision 46
# baseline (speedup 1.0000x reference)
"""Trainium2 Bass kernel for the MoE feed-forward block (nn_MoEFF).

Strategy: pure data-parallel over tokens. The 4096 tokens are split into
8 slices of 512; each NeuronCore runs the *entire* network on its slice
(router + all 8 experts dense-masked + shared expert). With E=8/K=4 every
expert serves ~half the tokens anyway, so dense-masked expert compute
costs only 2x the ideal sparse flops and avoids all collectives.

Layout: activations are kept transposed in SBUF ([feature-on-partition,
token-on-free]) so weight matrices in natural [in, out] layout are the
stationary matmul operand. lin0, swiglu1, the router, and the expert +
shared-expert FFNs all run in fp8 (e4m3) with DoubleRow perf mode
(256-deep contraction per matmul, 2x MACs per PE cycle); only the output
tail (lin1 + swiglu2) stays bf16, since its input-dependent signal is
small relative to its biases and fp8 noise there blows the error budget.
Measured end-to-end rel err ~7e-3 vs the fp32 reference (budget 2e-2):
the downstream swiglu blocks are bias-dominated and attenuate the
MoE-stage quantization error, and the ~12% of tokens whose top-4 expert
set flips under the quantized router land on near-tied experts.

All DRAM tensors are packed partition-major on the host ([128, free],
row p = the KT contraction chunks back-to-back) so DMAs use >=8KB
contiguous descriptors per partition (2KB descriptors ran at ~22 GB/s
aggregate; >=8KB runs near HBM rate). x and lin0 load in interleaved
halves so the first matmuls start after ~0.75MB of DMA, and the output
streams out in four chunks as swiglu2 m-tiles complete.

Measured on trn2: ~310us per-core NEFF span (308-312 run-to-run, cores
within +-0.5%), ~93% PE busy. The expert y-stage is software-pipelined
one expert behind g/u so the ACT/DVE v8 chain drains under PE work;
expert 1 is combine-free (cb applied in its y accumulate) so its g/u is
emitted before the router transposes; the router chain (no softmax
max-subtraction -- |z| << 1) is interleaved between expert-0 m-tiles so
DVE keeps pace with PSUM drains. Remaining span is the matmul stream
itself (1274 matmuls, 216ns median at N=512) plus ~6us startup DMA,
~4us output tail, and ~8us of semaphore micro-gaps.
"""

from contextlib import ExitStack

import ml_dtypes
import numpy as np

B, S, D = 2, 2048, 1024
E, TOPK, H = 8, 4, 1024
SH = 2 * H
NCORES = 8
T = B * S                 # 4096 tokens
TPC = T // NCORES         # 512 tokens per core
KT = D // 128             # 8 contraction tiles of 128
KT2 = KT // 2             # 4 DoubleRow contraction tiles of 256
MT_H = H // 128           # 8
MT_SH = SH // 128         # 16
NT = TPC // 128           # 4 token sub-tiles (router)

bf16 = ml_dtypes.bfloat16
f8 = ml_dtypes.float8_e4m3

# fp8 scales. e4m3 (IEEE variant) max normal is 240; absmax after scaling:
# weights 0.103*1024=105, x 5.06*32=162, h0 3.3*32=106, hbf 2.24*64=143,
# v 0.052*2048=106.
WS = 1024.0               # fp8 weight scale (all quantized weights)
XS = 32.0                 # x input scale
H0S = 32.0                # h0 activation scale
HS = 64.0                 # hbf activation scale
VS = 2048.0               # v = silu(g)*u scale
SG = 1.0 / (WS * HS)      # silu input descale (expert g path)
S1 = 1.0 / (WS * H0S)     # swiglu1 pa/pb descale
SL0 = H0S / (WS * XS)     # lin0 psum -> h0q scale (bias pre-scaled by H0S)
C_FOLD = VS / (WS * HS)   # u-path scale; folded into sel/combine
YS = 1.0 / (WS * VS)      # y accumulate descale
S0 = YS / C_FOLD          # expert-0 y-init descale (cb carries C_FOLD)

# bf16 weight units ([1024, 1024] each, packed to [128, KT*1024])
U_LIN1, U_S2A, U_S2B = 0, 1, 2
N_UNITS_BF = 3
# fp8 weight units: lin0/swi1 front, expert e -> 3+3e+{0:w1, 1:w3, 2:w2},
# shared at 27..32
U8_LIN0, U8_S1A, U8_S1B = 0, 1, 2
U8_EXP = 3
U8_SH1 = 27               # 27, 28: sh_w1 cols [0:1024], [1024:2048]
U8_SH3 = 29               # 29, 30
U8_SH2 = 31               # 31, 32: sh_w2 rows [0:1024], [1024:2048]
N_UNITS_F8 = 33

_prog = None  # built once per process
last_results = None  # BassKernelResults of the most recent kernel() call


def _build_program():
    import concourse.bacc as bacc
    import concourse.mybir as mybir
    import concourse.tile as tile

    F32, BF, F8 = mybir.dt.float32, mybir.dt.bfloat16, mybir.dt.float8e4
    AF = mybir.ActivationFunctionType
    OP = mybir.AluOpType
    DR = mybir.MatmulPerfMode.DoubleRow

    nc = bacc.Bacc()

    wall_d = nc.dram_tensor("wall", [N_UNITS_BF * 128, KT * 1024], BF, kind="ExternalInput")
    wall8_d = nc.dram_tensor("wall8", [N_UNITS_F8 * 128, KT * 1024], F8, kind="ExternalInput")
    xT_d = nc.dram_tensor("xT", [128, KT * TPC], F8, kind="ExternalInput")
    gate_d = nc.dram_tensor("gateT", [128, KT * E], F8, kind="ExternalInput")
    bias_d = nc.dram_tensor("biases", [128, 6 * KT], mybir.dt.float32, kind="ExternalInput")
    ident_d = nc.dram_tensor("ident", [128, 128], mybir.dt.float32, kind="ExternalInput")
    sel_d = nc.dram_tensor("sel", [E, E * 128], mybir.dt.float32, kind="ExternalInput")
    out_d = nc.dram_tensor("outT", [128, KT * TPC], mybir.dt.float32, kind="ExternalOutput")

    with tile.TileContext(nc) as tc, ExitStack() as ctx:
        wp = ctx.enter_context(tc.tile_pool(name="wp", bufs=3))
        wp8 = ctx.enter_context(tc.tile_pool(name="wp8", bufs=9))
        sp = ctx.enter_context(tc.tile_pool(name="sp", bufs=1))
        dp = ctx.enter_context(tc.tile_pool(name="dp", bufs=4))
        pp = ctx.enter_context(tc.tile_pool(name="pp", bufs=2, space="PSUM"))

        def wload(unit):
            wt = wp.tile([128, KT, 1024], BF, tag="wmat", name=f"w{unit}")
            nc.sync.dma_start(
                wt[:],
                wall_d[unit * 128:(unit + 1) * 128, :].rearrange(
                    "p (k c) -> p k c", k=KT
                ),
            )
            return wt

        def wload8(unit):
            wt = wp8.tile([128, KT, 1024], F8, tag="wmat8", name=f"w8_{unit}")
            nc.sync.dma_start(
                wt[:],
                wall8_d[unit * 128:(unit + 1) * 128, :].rearrange(
                    "p (k c) -> p k c", k=KT
                ),
            )
            return wt

        # ---- static inputs ----
        # x and lin0 are loaded in halves, interleaved, so the first lin0
        # matmuls start after ~0.75MB of DMA instead of ~1.5MB. HWDGE DMAs
        # drain in issue order per engine.
        # Two HWDGE rings (SP + ACT) drain in parallel: weights go on the
        # sync ring, activations/constants on the scalar ring.
        ident = sp.tile([128, 128], F32, tag="ident", name="ident")
        nc.scalar.dma_start(ident[:], ident_d[:])
        biases = sp.tile([128, 6 * KT], F32, tag="biases", name="biases")
        x8q, wl0q = [], []
        for q in range(4):  # interleave x / lin0 quarters, one tile each
            xq = sp.tile([128, 2, TPC], F8, tag="x8q", bufs=4, name=f"x8q{q}")
            nc.scalar.dma_start(
                xq[:],
                xT_d[:, 2 * q * TPC:(2 * q + 2) * TPC].rearrange(
                    "p (k t) -> p k t", k=2))
            x8q.append(xq)
            wq = wp8.tile([128, 2, 1024], F8, tag="wl0q", bufs=4, name=f"wl0q{q}")
            nc.sync.dma_start(
                wq[:],
                wall8_d[U8_LIN0 * 128:(U8_LIN0 + 1) * 128,
                        2 * q * 1024:(2 * q + 2) * 1024].rearrange(
                    "p (k c) -> p k c", k=2))
            wl0q.append(wq)
            if q == 0:
                nc.scalar.dma_start(biases[:], bias_d[:])

        # Warm the PE clock gate (HAM) during the startup DMA window so the
        # first real matmuls run at 2.4 GHz. The source tile is memset (no
        # DMA dependency) so warmup spans the whole ~11us window; the result
        # is never read.
        wsrc = sp.tile([128, 128], F32, tag="wsrc", name="wsrc")
        nc.vector.memset(wsrc[:], 1.0)
        warm = pp.tile([128, 128], F32, tag="misc", name="warm")
        for i in range(36):
            nc.tensor.matmul(warm[:], wsrc[:], wsrc[:],
                             start=(i == 0), stop=(i == 35))

        def bcol(idx, n):
            # per-partition bias column n of bias group idx
            return biases[:, idx * KT + n:idx * KT + n + 1]

        # ---- block 1: h0q = H0S*(x @ lin0 + b), fp8 DoubleRow ----
        # bias group 0 is pre-scaled by H0S on the host; w_lin0 loaded above.
        h0q = sp.tile([128, KT, TPC], F8, tag="h0q", name="h0q")
        for half in range(2):  # kk-major: first 4 matmuls need only quarter 0
            pss = []
            for n4 in range(4):
                n = half * 4 + n4
                ps = pp.tile([128, TPC], F32, tag="gu", bufs=4, name="ps_h0")
                pss.append((n, ps))
            for kk in range(KT2):
                for n, ps in pss:
                    nc.tensor.matmul(ps[:], wl0q[kk][:, :, n * 128:(n + 1) * 128],
                                     x8q[kk][:],
                                     start=(kk == 0), stop=(kk == KT2 - 1),
                                     perf_mode=DR)
            for n, ps in pss:
                nc.scalar.activation(h0q[:, n, :], ps[:], AF.Identity,
                                     bias=bcol(0, n), scale=SL0)

        # ---- swiglu 1 (fp8 DR) -> hbf8 (fp8; feeds experts AND router) ----
        # bias group 2 is pre-scaled by HS on the host.
        w_s1a, w_s1b = wload8(U8_S1A), wload8(U8_S1B)
        gsb = sp.tile([128, KT, E], F8, tag="gsb", name="gsb")
        nc.scalar.dma_start(gsb[:], gate_d[:].rearrange("p (k e) -> p k e", k=KT))
        sel = sp.tile([E, E * 128], F32, tag="sel", name="sel")
        nc.scalar.dma_start(sel[:], sel_d[:])
        hbf8 = sp.tile([128, KT, TPC], F8, tag="hbf8", name="hbf8")
        for m in range(KT):
            pa = pp.tile([128, TPC], F32, tag="gu", bufs=4, name="ps_a1")
            for kk in range(KT2):
                nc.tensor.matmul(pa[:], w_s1a[:, 2 * kk:2 * kk + 2, m * 128:(m + 1) * 128],
                                 h0q[:, 2 * kk:2 * kk + 2, :],
                                 start=(kk == 0), stop=(kk == KT2 - 1), perf_mode=DR)
            pb = pp.tile([128, TPC], F32, tag="gu", bufs=4, name="ps_b1")
            for kk in range(KT2):
                nc.tensor.matmul(pb[:], w_s1b[:, 2 * kk:2 * kk + 2, m * 128:(m + 1) * 128],
                                 h0q[:, 2 * kk:2 * kk + 2, :],
                                 start=(kk == 0), stop=(kk == KT2 - 1), perf_mode=DR)
            sa = dp.tile([128, TPC], F32, tag="gs", bufs=2, name="sa1")
            nc.scalar.activation(sa[:], pa[:], AF.Silu, bias=bcol(1, m), scale=S1)
            ub = dp.tile([128, TPC], F32, tag="v", bufs=2, name="ub1")
            nc.vector.tensor_scalar(ub[:], pb[:], S1 * HS, bcol(2, m), OP.mult, OP.add)
            nc.vector.tensor_mul(hbf8[:, m, :], ub[:], sa[:])

        # ---- router matmuls: z[t, e] (fp8; z = HS*WS*z_true) ----
        z_all = pp.tile([128, NT * E], F32, tag="misc", bufs=2, name="z_all")
        for t in range(NT):
            for k in range(KT):
                nc.tensor.matmul(z_all[:, t * E:(t + 1) * E],
                                 hbf8[:, k, t * 128:(t + 1) * 128],
                                 gsb[:, k, :], start=(k == 0), stop=(k == KT - 1))

        # ---- router chain (DVE/ACT; overlaps expert-0 g/u matmuls on PE) ----
        ez = sp.tile([128, NT * E], F32, tag="ez", name="ez")
        cur = sp.tile([128, NT * E], F32, tag="cur", name="cur")
        cm = sp.tile([128, NT * E], F32, tag="cm", name="cm")
        combine = sp.tile([128, NT * E], F32, tag="combine", name="combine")
        stat = sp.tile([128, 4 * NT], F32, tag="stat", name="stat")  # nmx, thr, s, r

        SZ = 1.0 / (HS * WS)  # z descale for the softmax exp

        def chain_t(t):
            # Per-token-subtile router chain. No max-subtraction: |z_true| is
            # well under 1, exp cannot overflow, and softmax normalizes anyway.
            zt = z_all[:, t * E:(t + 1) * E]
            ezt = ez[:, t * E:(t + 1) * E]
            nc.scalar.activation(ezt, zt, AF.Exp, scale=SZ)
            curt = cur[:, t * E:(t + 1) * E]
            nc.vector.tensor_copy(curt, ezt)
            thr = stat[:, NT + t:NT + t + 1]
            for i in range(TOPK):
                nc.vector.tensor_reduce(thr, curt, mybir.AxisListType.X, OP.max)
                if i < TOPK - 1:
                    eq = dp.tile([128, E], F32, tag="eq", bufs=2, name="eq")
                    nc.vector.tensor_scalar(eq[:], curt, thr, None, OP.is_equal)
                    nc.vector.scalar_tensor_tensor(curt, eq[:], -1e30, curt,
                                                   OP.mult, OP.add)
            cmt = cm[:, t * E:(t + 1) * E]
            # cm = ez * (ez >= thr); reuse cur as the mask buffer
            nc.vector.tensor_scalar(curt, ezt, thr, None, OP.is_ge)
            nc.vector.tensor_mul(cmt, ezt, curt)
            s = stat[:, 2 * NT + t:2 * NT + t + 1]
            nc.vector.tensor_reduce(s, cmt, mybir.AxisListType.X, OP.add)
            r = stat[:, 3 * NT + t:3 * NT + t + 1]
            nc.vector.reciprocal(r, s)
            nc.vector.tensor_scalar(combine[:, t * E:(t + 1) * E], cmt, r, None,
                                    OP.mult)

        cbT = sp.tile([E, TPC], F32, tag="cbT", name="cbT")

        def emit_transposes():
            for t in range(NT):
                trp = pp.tile([E, 128], F32, tag="misc", name="trp")
                nc.tensor.transpose(trp[:], combine[:, t * E:(t + 1) * E], ident[:])
                nc.scalar.activation(cbT[0:E, t * 128:(t + 1) * 128], trp[:], AF.Copy)

        def outer(e):
            # cb_ps[p, t] = sum_k sel[k, e*128+p] * cbT[k, t] = C_FOLD*combine[t, e]
            # (sel carries C_FOLD from the host).
            cb_ps = pp.tile([128, TPC], F32, tag="misc", name="cb_ps")
            nc.tensor.matmul(cb_ps[:], sel[:, e * 128:(e + 1) * 128], cbT[0:E, :],
                             start=True, stop=True)
            return cb_ps

        def emit_gu(w1, w3, n_m, cb_ps, v8, after_m=None):
            """DoubleRow g/u for one expert; writes fp8 v8 slices.
            v8[:, m, :] = VS * silu(g)*u * [combine] (scales pre-folded).
            after_m: {m: fn} emitted after tile m (router-chain interleave)."""
            for m in range(n_m):
                u, mm = divmod(m, KT)
                pg = pp.tile([128, TPC], F32, tag="gu", bufs=4, name="ps_g")
                for kk in range(KT2):
                    nc.tensor.matmul(pg[:], w1[u][:, 2 * kk:2 * kk + 2, mm * 128:(mm + 1) * 128],
                                     hbf8[:, 2 * kk:2 * kk + 2, :],
                                     start=(kk == 0), stop=(kk == KT2 - 1), perf_mode=DR)
                pu = pp.tile([128, TPC], F32, tag="gu", bufs=4, name="ps_u")
                for kk in range(KT2):
                    nc.tensor.matmul(pu[:], w3[u][:, 2 * kk:2 * kk + 2, mm * 128:(mm + 1) * 128],
                                     hbf8[:, 2 * kk:2 * kk + 2, :],
                                     start=(kk == 0), stop=(kk == KT2 - 1), perf_mode=DR)
                gs = dp.tile([128, TPC], F32, tag="gs", bufs=2, name="gs")
                nc.scalar.activation(gs[:], pg[:], AF.Silu, scale=SG)
                if cb_ps is None:
                    nc.vector.scalar_tensor_tensor(v8[:, m, :], pu[:], C_FOLD, gs[:],
                                                   OP.mult, OP.mult)
                else:
                    v = dp.tile([128, TPC], F32, tag="v", bufs=2, name="v")
                    nc.vector.tensor_mul(v[:], gs[:], pu[:])
                    nc.vector.tensor_mul(v8[:, m, :], v[:], cb_ps[:])
                if after_m and m in after_m:
                    after_m[m]()

        def emit_y(w2, v8, n_m, acc, cb_sb=None, cb_acc=None):
            """DoubleRow y = v8 @ w2, descaled into acc (bf16 SBUF).
            cb_sb: init acc with py*S0*cb. cb_acc: acc += py*S0*cb (2 ops)."""
            n_planes = n_m // 2
            for n in range(KT):
                py = pp.tile([128, TPC], F32, tag="y", bufs=2, name="ps_y")
                for j in range(n_planes):
                    u, jj = divmod(j, KT2)
                    nc.tensor.matmul(py[:], w2[u][:, 2 * jj:2 * jj + 2, n * 128:(n + 1) * 128],
                                     v8[u][:, 2 * jj:2 * jj + 2, :],
                                     start=(j == 0), stop=(j == n_planes - 1), perf_mode=DR)
                if cb_sb is not None:
                    a = sp.tile([128, TPC], BF, tag="acc", bufs=8, name=f"acc_{n}")
                    nc.vector.scalar_tensor_tensor(a[:], py[:], S0, cb_sb[:],
                                                   OP.mult, OP.mult)
                    acc.append(a)
                elif cb_acc is not None:
                    yt = dp.tile([128, TPC], F32, tag="v", bufs=2, name="yt")
                    nc.vector.scalar_tensor_tensor(yt[:], py[:], S0, cb_acc[:],
                                                   OP.mult, OP.mult)
                    nc.vector.tensor_add(acc[n][:], acc[n][:], yt[:])
                else:
                    nc.vector.scalar_tensor_tensor(acc[n][:], py[:], YS, acc[n][:],
                                                   OP.mult, OP.add)

        # ---- experts (fp8 DoubleRow) ----
        # Software-pipelined by one stage: expert e's y matmuls are emitted
        # after expert e+1's g/u matmuls, so the ACT+DVE chain producing the
        # last v8 slices drains under PE work instead of stalling PE.
        acc = []
        pend_y = None  # (w2 units, v8 tile, n_m, cb_sb, cb_acc)
        for e in range(E):
            we1 = wload8(U8_EXP + 3 * e)
            we3 = wload8(U8_EXP + 3 * e + 1)
            we2 = wload8(U8_EXP + 3 * e + 2)
            v8 = dp.tile([128, MT_H, TPC], F8, tag="v8e", bufs=3, name=f"v8_{e}")
            if e == 0:
                emit_gu([we1], [we3], MT_H, None, v8,
                        after_m={1: lambda: chain_t(0), 3: lambda: chain_t(1),
                                 5: lambda: chain_t(2), 7: lambda: chain_t(3)})
                pend_y = ([we2], [v8], MT_H, None, None)  # cb via cb_sb0 below
            elif e == 1:
                # combine-free gu emitted BEFORE the transposes: PE chews
                # expert 1 while the router chain (DVE) finishes; cb is
                # applied in this expert's y-stage instead.
                emit_gu([we1], [we3], MT_H, None, v8)
                emit_transposes()
                cb_ps0 = outer(0)
                cb_sb0 = dp.tile([128, TPC], F32, tag="cbsb", bufs=2, name="cb_sb0")
                nc.scalar.activation(cb_sb0[:], cb_ps0[:], AF.Copy)
                cb_ps1 = outer(1)
                cb_sb1 = dp.tile([128, TPC], F32, tag="cbsb", bufs=2, name="cb_sb1")
                nc.scalar.activation(cb_sb1[:], cb_ps1[:], AF.Copy)
                w2p, v8p, nmp, _, _ = pend_y
                emit_y(w2p, v8p, nmp, acc, cb_sb=cb_sb0)
                pend_y = ([we2], [v8], MT_H, None, cb_sb1)
            else:
                cb_ps = outer(e)
                emit_gu([we1], [we3], MT_H, cb_ps, v8)
                w2p, v8p, nmp, cbp, cbap = pend_y
                emit_y(w2p, v8p, nmp, acc, cb_sb=cbp, cb_acc=cbap)
                pend_y = ([we2], [v8], MT_H, None, None)

        # ---- shared expert (always-on, unscaled) ----
        sh1 = [wload8(U8_SH1), wload8(U8_SH1 + 1)]
        sh3 = [wload8(U8_SH3), wload8(U8_SH3 + 1)]
        sh2 = [wload8(U8_SH2), wload8(U8_SH2 + 1)]
        v8sA = dp.tile([128, MT_H, TPC], F8, tag="v8e", bufs=3, name="v8_shA")
        v8sB = dp.tile([128, MT_H, TPC], F8, tag="v8e", bufs=3, name="v8_shB")
        emit_gu([sh1[0]], [sh3[0]], MT_H, None, v8sA)
        w2p, v8p, nmp, cbp, cbap = pend_y
        emit_y(w2p, v8p, nmp, acc, cb_sb=cbp, cb_acc=cbap)
        emit_gu([sh1[1]], [sh3[1]], MT_H, None, v8sB)
        emit_y(sh2, [v8sA, v8sB], MT_SH, acc)

        # ---- block 3: lin1 + swiglu2 (bf16; acc is already bf16) ----
        accbf = acc
        w_lin1 = wload(U_LIN1)
        h2 = []
        for n in range(KT):
            ps = pp.tile([128, TPC], F32, tag="gu", bufs=4, name="ps_h2")
            for k in range(KT):
                nc.tensor.matmul(ps[:], w_lin1[:, k, n * 128:(n + 1) * 128],
                                 accbf[k][:], start=(k == 0), stop=(k == KT - 1))
            t = sp.tile([128, TPC], BF, tag="hbf", bufs=8, name=f"h2_{n}")
            nc.scalar.activation(t[:], ps[:], AF.Identity, bias=bcol(3, n))
            h2.append(t)

        w_s2a, w_s2b = wload(U_S2A), wload(U_S2B)
        osb = sp.tile([128, KT, TPC], F32, tag="osb", name="osb")
        for m in range(KT):
            pa = pp.tile([128, TPC], F32, tag="gu", bufs=4, name="ps_a2")
            for k in range(KT):
                nc.tensor.matmul(pa[:], w_s2a[:, k, m * 128:(m + 1) * 128],
                                 h2[k][:], start=(k == 0), stop=(k == KT - 1))
            pb = pp.tile([128, TPC], F32, tag="gu", bufs=4, name="ps_b2")
            for k in range(KT):
                nc.tensor.matmul(pb[:], w_s2b[:, k, m * 128:(m + 1) * 128],
                                 h2[k][:], start=(k == 0), stop=(k == KT - 1))
            sa = dp.tile([128, TPC], F32, tag="gs", bufs=2, name="sa2")
            nc.scalar.activation(sa[:], pa[:], AF.Silu, bias=bcol(4, m))
            nc.vector.scalar_tensor_tensor(osb[:, m, :], pb[:], bcol(5, m), sa[:],
                                           OP.add, OP.mult)
            if m in (2, 5, 6):  # stream out; last chunks small
                lo = {2: 0, 5: 3, 6: 6}[m]
                nc.sync.dma_start(
                    out_d[:, lo * TPC:(m + 1) * TPC],
                    osb[:, lo:m + 1, :].rearrange("p k t -> p (k t)"))
            elif m == 7:  # last tile: halves on both rings in parallel
                nc.sync.dma_start(out_d[:, 7 * TPC:7 * TPC + 256],
                                  osb[:, 7, 0:256])
                nc.scalar.dma_start(out_d[:, 7 * TPC + 256:8 * TPC],
                                  osb[:, 7, 256:512])

    # run_bass_via_pjrt serializes the BIR as-is; Bacc's lowering passes
    # (register allocation, TRN2 single-wait splitting) only run in
    # finalize(), so it must happen before dispatch.
    nc.finalize()
    return nc


def _pack_rows(w):
    # [1024(in), C] -> [128, KT*C]: row p holds the KT contraction chunks
    # (rows k*128+p) back-to-back -> 16KB/8KB contiguous per partition.
    c = w.shape[1]
    return w.reshape(KT, 128, c).transpose(1, 0, 2).reshape(128, KT * c)


def _pack_weights(inp):
    def b(a):
        w = np.ascontiguousarray(np.asarray(a, dtype=np.float32)).astype(bf16)
        return _pack_rows(w)

    units = [b(inp["lin1_w"]), b(inp["swi2_w1"]), b(inp["swi2_w2"])]
    assert len(units) == N_UNITS_BF
    return np.ascontiguousarray(np.concatenate(units, axis=0))


def _pack_weights_f8(inp):
    def q(a):
        w = (np.asarray(a, np.float32) * WS).astype(f8)
        return _pack_rows(w)

    units = [q(inp["lin0_w"]), q(inp["swi1_w1"]), q(inp["swi1_w2"])]
    w1, w3, w2 = (np.asarray(inp["exp_w1"], np.float32),
                  np.asarray(inp["exp_w3"], np.float32),
                  np.asarray(inp["exp_w2"], np.float32))
    for e in range(E):
        units += [q(w1[e]), q(w3[e]), q(w2[e])]
    sh1 = np.asarray(inp["sh_w1"], np.float32)
    sh3 = np.asarray(inp["sh_w3"], np.float32)
    sh2 = np.asarray(inp["sh_w2"], np.float32)
    units += [q(sh1[:, :1024]), q(sh1[:, 1024:]),
              q(sh3[:, :1024]), q(sh3[:, 1024:]),
              q(sh2[:1024, :]), q(sh2[1024:, :])]
    assert len(units) == N_UNITS_F8
    return np.ascontiguousarray(np.concatenate(units, axis=0))


def _pack_biases(inp):
    cols = []
    for name, sc in [("lin0_b", H0S), ("swi1_b1", 1.0), ("swi1_b2", HS),
                     ("lin1_b", 1.0), ("swi2_b1", 1.0), ("swi2_b2", 1.0)]:
        v = (np.asarray(inp[name], np.float32) * sc).reshape(KT, 128).T
        cols.append(v)
    return np.ascontiguousarray(np.concatenate(cols, axis=1))  # [128, 6*KT]


def kernel(**inputs):
    global _prog
    from concourse.bass_utils import run_bass_kernel_spmd

    if _prog is None:
        _prog = _build_program()
    nc = _prog

    wall = _pack_weights(inputs)
    wall8 = _pack_weights_f8(inputs)
    biases = _pack_biases(inputs)
    gateT = (np.asarray(inputs["gate_w"], np.float32).T * WS).astype(f8)  # [D, E]
    gateT = np.ascontiguousarray(
        gateT.reshape(KT, 128, E).transpose(1, 0, 2).reshape(128, KT * E))
    ident = np.eye(128, dtype=np.float32)
    sel = np.zeros((E, E * 128), dtype=np.float32)
    for e in range(E):
        sel[e, e * 128:(e + 1) * 128] = C_FOLD

    x = np.asarray(inputs["x"], np.float32).reshape(T, D)
    in_maps = []
    for c in range(NCORES):
        xT = (x[c * TPC:(c + 1) * TPC, :].T * XS).astype(f8)  # [D, TPC]
        xTp = np.ascontiguousarray(
            xT.reshape(KT, 128, TPC).transpose(1, 0, 2).reshape(128, KT * TPC))
        in_maps.append({
            "wall": wall, "wall8": wall8, "xT": xTp, "gateT": gateT,
            "biases": biases, "ident": ident, "sel": sel,
        })

    res = run_bass_kernel_spmd(nc, in_maps, list(range(NCORES)))
    global last_results
    last_results = res
    # outT per core: [128, KT*TPC], row p = concat_k y_T[k*128+p, :]
    outs = []
    for c in range(NCORES):
        o = res.results[c]["outT"].reshape(128, KT, TPC).transpose(1, 0, 2)
        outs.append(o.reshape(D, TPC))
    outT = np.concatenate(outs, axis=1)
    return np.ascontiguousarray(outT.T).reshape(B, S, D).astype(np.float32)
